# revision 29
# baseline (speedup 1.0000x reference)
"""Causal multi-head attention (B=2, S=2048, D=1024, H=16) on 8 trn2 cores.

Sharding: batch (2-way) x head-group (4-way) = 8 cores. Each core computes
QKV projection for its batch restricted to its 4 heads, causal attention,
and a row-parallel slice of the output projection; the host sums the 4
partial outputs per batch (the all-reduce of the row-parallel Wo matmul).

Per-core kernel (Tile framework, fp16 matmul operands / fp32 PSUM accum):
  - The host ships x pre-transposed ([D, S] fp16) and the weight slices in
    fp16, so contraction dims land on SBUF partitions with plain DMAs.
  - Q,K are produced in [feat, seq] layout (rhs = x^T), V in [seq, feat]
    layout (lhsT = x^T) with an extra ones-column per head so the PV matmul
    also produces the softmax denominator.
  - Scores are computed transposed, S_T[key, q] = K_blk.T @ Q. The two heads
    of a pair live on partitions 0:64 / 64:128, so their K=64 score matmuls
    map to distinct PE row-groups and run concurrently; emission interleaves
    j2-outer/head-inner to keep the pairs adjacent.
  - exp on ScalarE (scale folded in), one activation per (head, key-block
    pair) covering the full live column range; causal staircase masking via
    gpsimd.affine_select directly on the exp output (fully-masked column
    prefixes are never read by PV and stay garbage).
  - PV: out_T[65, q] = V_aug.T @ exp(S_T), accumulated over key blocks; row
    64 is the denominator. Normalization batches both heads of a pair: one
    [2,512] reciprocal, one K=2 fp32r broadcast matmul into [128,512].
  - Wo: out[q, :] = sum_c vw_T_c.T @ Wo_c; partials leave as fp16 via
    gpsimd PSUM->SBUF copies; host sums partials and adds bo in fp32.
"""

import numpy as np
import ml_dtypes
from contextlib import ExitStack

import concourse.bass as bass
import concourse.mybir as mybir
import concourse.tile as tile
from concourse import bacc
from concourse.bass_utils import run_bass_kernel_spmd

B, S, D, H, HD = 2, 2048, 1024, 16, 64
NCORES = 8
NHG = 4                  # head groups (cores per batch)
NH = H // NHG            # 4 local heads
FQK = NH * HD * 2        # 512 local q+k features
FV = NH * HD             # 256 local v features
QB = 512                 # query block (attention outer tile)
KB = 128                 # key block
NSC = S // QB            # 4 seq chunks
R32 = mybir.dt.float32r
F16 = mybir.dt.float16
F32 = mybir.dt.float32
F8 = mybir.dt.float8e4
DR = mybir.MatmulPerfMode.DoubleRow
EXP = mybir.ActivationFunctionType.Exp
GE = mybir.AluOpType.is_ge
# Wqk is pre-scaled by WSCALE host-side so fp8e4 quantization stays out of
# the subnormal range (raw std 0.002); folded back out in the exp scale.
WSCALE = 64.0


def _build_body(ctx, tc, x_d, x8_d, wqk8_d, wv_d, bqk_d, bv_d, wo_d, out_d):
    nc = tc.nc

    const = ctx.enter_context(tc.tile_pool(name="const", bufs=1))
    wq_pool = ctx.enter_context(tc.tile_pool(name="wqp", bufs=8))
    wvp = ctx.enter_context(tc.tile_pool(name="wvp", bufs=8))
    wop = ctx.enter_context(tc.tile_pool(name="wop", bufs=2))
    xt_pool = ctx.enter_context(tc.tile_pool(name="xtp", bufs=16))
    x8_pool = ctx.enter_context(tc.tile_pool(name="x8p", bufs=16))
    qk_pool = ctx.enter_context(tc.tile_pool(name="qkp", bufs=1))
    v_pool = ctx.enter_context(tc.tile_pool(name="vp", bufs=16))
    exp_pool = ctx.enter_context(tc.tile_pool(name="ep", bufs=6))
    vw_pool = ctx.enter_context(tc.tile_pool(name="vwp", bufs=2))
    rc_pool = ctx.enter_context(tc.tile_pool(name="rcp", bufs=3))
    os_pool = ctx.enter_context(tc.tile_pool(name="osp", bufs=3))
    p1 = ctx.enter_context(tc.tile_pool(name="p1", bufs=2, space="PSUM"))
    ps = ctx.enter_context(tc.tile_pool(name="ps", bufs=2, space="PSUM"))
    po = ctx.enter_context(tc.tile_pool(name="po", bufs=2, space="PSUM"))



    # ---- weights ----
    # bqk/bv first: tiny, and the bvb broadcast matmul is the first PE
    # instruction - queued behind the bulk weights it stalls the PE stream
    bqk_sb = const.tile([128, 4], F32)
    nc.sync.dma_start(bqk_sb, bqk_d.ap().rearrange("(f p) -> p f", p=128))
    bv_sb = const.tile([1, FV], F32)
    nc.sync.dma_start(bv_sb, bv_d.ap().rearrange("(o e) -> o e", o=1))
    # fp8 DoubleRow weights: [c2][p, j, f] holds Wqk[c2*256 + j*128 + p, f]
    wqk8_sb = []
    for c2 in range(4):
        t = wq_pool.tile([128, 2, FQK], F8, name=f"wqk{c2}", tag="wqk")
        nc.sync.dma_start(
            t, wqk8_d.ap()[c2 * 128:(c2 + 1) * 128, :].rearrange(
                "p (j f) -> p j f", j=2))
        wqk8_sb.append(t)
    # x8 chunk 0 before the remaining weights: the first QK-proj matmuls
    # need only wqk8 + x8
    x8_0 = []
    for c2 in range(4):
        xt = x8_pool.tile([128, 2, QB], F8, name="x8", tag="x8")
        nc.sync.dma_start(
            xt, x8_d.ap()[c2 * 128:(c2 + 1) * 128, :].rearrange(
                "p (j s) -> p j s", j=2)[:, :, 0:QB])
        x8_0.append(xt)
    wv_sb = []
    for dc in range(8):
        t = wvp.tile([128, FV], F16, name=f"wv{dc}", tag="wv")
        nc.sync.dma_start(t, wv_d.ap()[dc * 128:(dc + 1) * 128, :])
        wv_sb.append(t)
    xT0 = []
    for dc in range(8):
        xt = xt_pool.tile([128, QB], F16, name="xt", tag="xt")
        nc.sync.dma_start(xt, x_d.ap()[dc * 128:(dc + 1) * 128, 0:QB])
        xT0.append(xt)
    wo_sb = []
    for c in range(2):
        t = wop.tile([128, D], F16, name=f"wo{c}", tag="wo")
        nc.sync.dma_start(t, wo_d.ap()[c * 128:(c + 1) * 128, :])
        wo_sb.append(t)
    # v-bias broadcast across partitions on gpsimd (SBUF->SBUF)
    bvb_sb = const.tile([128, FV], F32)
    nc.gpsimd.partition_broadcast(bvb_sb, bv_sb)

    # ---- phase B: QKV projection ----
    qkT = [qk_pool.tile([128, S], F16, name=f"qkT{f}", tag=f"qkT{f}", bufs=1)
           for f in range(4)]
    v_tiles = []

    def emit_B(sc):
        # x tiles: fp8 interleaved for QK proj, fp16 transposed for V proj
        if sc == 0:
            x8, xT = x8_0, xT0
        else:
            x8 = []
            for c2 in range(4):
                xt = x8_pool.tile([128, 2, QB], F8, name="x8", tag="x8")
                nc.sync.dma_start(
                    xt, x8_d.ap()[c2 * 128:(c2 + 1) * 128, :].rearrange(
                        "p (j s) -> p j s", j=2)[:, :, sc * QB:(sc + 1) * QB])
                x8.append(xt)
            xT = []
            for dc in range(8):
                xt = xt_pool.tile([128, QB], F16, name="xt", tag="xt")
                nc.sync.dma_start(
                    xt,
                    x_d.ap()[dc * 128:(dc + 1) * 128, sc * QB:(sc + 1) * QB])
                xT.append(xt)
        # Q,K in [feat, seq]: psum += Wqk_c2.T @ x8, fp8 DoubleRow (K=256)
        for f in range(4):
            pq = p1.tile([128, QB], F32, name="pq", tag="p1")
            for c2 in range(4):
                nc.tensor.matmul(pq, wqk8_sb[c2][:, :, f * 128:(f + 1) * 128],
                                 x8[c2], start=(c2 == 0), stop=(c2 == 3),
                                 perf_mode=DR)
            nc.vector.tensor_scalar_add(
                qkT[f][:, sc * QB:(sc + 1) * QB], pq, bqk_sb[:, f:f + 1])
        # V in [seq, feat]: psum += (x^T_blk).T @ Wv_chunk, plus ones column
        for sb in range(4):
            pv = p1.tile([128, FV], F32, name="pv", tag="p1")
            for dc in range(8):
                nc.tensor.matmul(pv, xT[dc][:, sb * 128:(sb + 1) * 128],
                                 wv_sb[dc], start=(dc == 0), stop=(dc == 7))
            vt = v_pool.tile([128, NH, HD + 1], F16, name="vt", tag="vt")
            nc.vector.tensor_add(vt[:, :, 0:HD],
                                 pv.rearrange("p (h e) -> p h e", h=NH),
                                 bvb_sb.rearrange("p (h e) -> p h e", h=NH))
            nc.gpsimd.memset(vt[:, :, HD:HD + 1], 1.0)
            v_tiles.append(vt)

    def emit_C(qi):
        # ---- attention + output projection for query chunk qi ----
        vwT = [vw_pool.tile([128, QB], F16, name=f"vwT{c}", tag=f"vwT{c}")
               for c in range(2)]
        for hp in range(2):
            pair = (2 * hp, 2 * hp + 1)
            nkb = (qi + 1) * 4
            poh, Q, Kt = {}, {}, {}
            for h in pair:
                poh[h] = po.tile([HD + 1, QB], F32, name="poh", tag="po")
                r0 = (h % 2) * 64
                Q[h] = qkT[h // 2][r0:r0 + 64, qi * QB:(qi + 1) * QB]
                Kt[h] = qkT[2 + h // 2][r0:r0 + 64, :]

            def koff(kb):
                # columns q < (kb - qi*4)*128 of a diagonal key-block are
                # fully masked: skip them in scores/exp/PV
                return max(0, (kb - qi * 4)) * KB

            for base in range(0, nkb, 2):
                diag = base >= qi * 4
                o0 = koff(base)
                psn = {h: ps.tile([128, 2 * QB], F32, name="psn", tag="ps")
                       for h in pair}
                # j2-outer / head-inner: adjacent matmuls hit distinct PE
                # row-groups (partitions 0:64 vs 64:128) and run concurrently
                for j2 in range(2):
                    kb = base + j2
                    off = koff(kb)
                    for h in pair:
                        nc.tensor.matmul(
                            psn[h][:, j2 * QB + off:(j2 + 1) * QB],
                            Kt[h][:, kb * KB:(kb + 1) * KB],
                            Q[h][:, off:QB], start=True, stop=True)
                es = {}
                for h in pair:
                    e = exp_pool.tile([128, 2 * QB], F16, name="et", tag="et")
                    nc.scalar.activation(e[:, o0:2 * QB], psn[h][:, o0:2 * QB],
                                         EXP,
                                         scale=1.0 / (np.sqrt(HD) * WSCALE**2))
                    if diag:
                        # causal staircase: keep col q' >= partition k within
                        # each live [off:QB] slice (off == 128*j exactly)
                        for j2 in range(2):
                            off = koff(base + j2)
                            nc.gpsimd.affine_select(
                                out=e[:, j2 * QB + off:(j2 + 1) * QB],
                                in_=e[:, j2 * QB + off:(j2 + 1) * QB],
                                compare_op=GE, fill=0.0, base=0,
                                channel_multiplier=-1,
                                pattern=[[1, QB - off]])
                    es[h] = e
                for j2 in range(2):
                    kb = base + j2
                    off = koff(kb)
                    for h in pair:
                        nc.tensor.matmul(
                            poh[h][:, off:QB], v_tiles[kb][:, h, :],
                            es[h][:, j2 * QB + off:(j2 + 1) * QB],
                            start=(kb == 0), stop=(kb == nkb - 1))
            # normalization, both heads batched on one partition: one
            # reciprocal over [1, 2QB], then two col-tiled K=1 broadcast
            # matmuls (out partition bases 0 / 64) into one PSUM tile
            sum2 = rc_pool.tile([1, 2 * QB], F32, name="sum2", tag="sum2")
            for i, h in enumerate(pair):
                nc.vector.tensor_copy(sum2[:, i * QB:(i + 1) * QB],
                                      poh[h][HD:HD + 1, :])
            rc2 = rc_pool.tile([1, 2 * QB], F32, name="rc2", tag="rc2")
            nc.vector.reciprocal_approx_fast(rc2, sum2)
            for i, h in enumerate(pair):
                bcs = rc_pool.tile([64, QB], F32, name="bcs", tag="bcs")
                nc.gpsimd.partition_broadcast(bcs, rc2[:, i * QB:(i + 1) * QB])
                nc.vector.tensor_mul(vwT[hp][i * 64:(i + 1) * 64, :],
                                     poh[h][0:HD, :], bcs)
        for ql in range(4):
            for do in range(2):
                pw = p1.tile([128, QB], F32, name="pw", tag="p1")
                for c in range(2):
                    nc.tensor.matmul(pw, vwT[c][:, ql * 128:(ql + 1) * 128],
                                     wo_sb[c][:, do * QB:(do + 1) * QB],
                                     start=(c == 0), stop=(c == 1))
                osb = os_pool.tile([128, QB], F16, name="osb", tag="osb")
                nc.vector.tensor_copy(osb, pw)
                nc.sync.dma_start(
                    out_d.ap()[qi * QB + ql * 128: qi * QB + (ql + 1) * 128,
                               do * QB:(do + 1) * QB], osb)

    # interleave: C(qi) only needs K/V chunks 0..qi, so the projection PE
    # work of later chunks hides under the ScalarE exp stream of attention
    for sc in range(NSC):
        emit_B(sc)
        emit_C(sc)


_COMPILED = None


def get_compiled():
    global _COMPILED
    if _COMPILED is not None:
        return _COMPILED
    nc = bacc.Bacc("TRN2", target_bir_lowering=False, debug=False,
                   enable_asserts=False, num_devices=NCORES)
    x_d = nc.dram_tensor("x", [D, S], F16, kind="ExternalInput")
    x8_d = nc.dram_tensor("x8", [D // 2, 2 * S], F8, kind="ExternalInput")
    wqk8_d = nc.dram_tensor("wqk8", [D // 2, 2 * FQK], F8,
                            kind="ExternalInput")
    wv_d = nc.dram_tensor("wv", [D, FV], F16, kind="ExternalInput")
    bqk_d = nc.dram_tensor("bqk", [FQK], F32, kind="ExternalInput")
    bv_d = nc.dram_tensor("bv", [FV], F32, kind="ExternalInput")
    wo_d = nc.dram_tensor("wo", [FV, D], F16, kind="ExternalInput")
    out_d = nc.dram_tensor("out", [S, D], F16, kind="ExternalOutput")
    with tile.TileContext(nc) as tc:
        with ExitStack() as ctx:
            _build_body(ctx, tc, x_d, x8_d, wqk8_d, wv_d, bqk_d, bv_d, wo_d,
                        out_d)
    nc.compile()
    _COMPILED = nc
    return nc


def _pack_dr(a):
    """[D, cols] -> [D/2, 2*cols] fp8 with virtual row (p,j) of 256-chunk c2
    holding row c2*256 + j*128 + p (must match the kernel's rearrange)."""
    c = a.reshape(4, 2, 128, a.shape[1])
    c = np.ascontiguousarray(c.transpose(0, 2, 1, 3))
    return c.reshape(D // 2, 2 * a.shape[1]).astype(ml_dtypes.float8_e4m3)


def make_in_maps(x, Wqkv, bqkv, Wo):
    x = np.ascontiguousarray(np.asarray(x, dtype=np.float32))
    Wqkv = np.asarray(Wqkv, dtype=np.float32)
    bqkv = np.asarray(bqkv, dtype=np.float32)
    Wo = np.asarray(Wo, dtype=np.float32)
    in_maps = []
    xT = [np.ascontiguousarray(x[b].T) for b in range(B)]
    x8 = [_pack_dr(t) for t in xT]
    for c in range(NCORES):
        b, hg = divmod(c, NHG)
        qs = slice(hg * FV, (hg + 1) * FV)
        ks = slice(D + hg * FV, D + (hg + 1) * FV)
        vs = slice(2 * D + hg * FV, 2 * D + (hg + 1) * FV)
        wqk = np.concatenate([Wqkv[:, qs], Wqkv[:, ks]], axis=1)
        in_maps.append({
            "x": xT[b].astype(np.float16),
            "x8": x8[b],
            "wqk8": _pack_dr(wqk * WSCALE),
            "wv": np.ascontiguousarray(Wqkv[:, vs]).astype(np.float16),
            "bqk": np.ascontiguousarray(
                np.concatenate([bqkv[qs], bqkv[ks]])) * WSCALE,
            "bv": np.ascontiguousarray(bqkv[vs]),
            "wo": np.ascontiguousarray(Wo[hg * FV:(hg + 1) * FV, :]).astype(np.float16),
        })
    return in_maps


def run_sharded(x, Wqkv, bqkv, Wo, bo, **spmd_kwargs):
    nc = get_compiled()
    in_maps = make_in_maps(x, Wqkv, bqkv, Wo)
    res = run_bass_kernel_spmd(nc, in_maps, core_ids=list(range(NCORES)),
                               **spmd_kwargs)
    out = np.zeros((B, S, D), np.float32)
    for c in range(NCORES):
        out[c // NHG] += res.results[c]["out"].astype(np.float32)
    out += np.asarray(bo, dtype=np.float32)
    return out, res


def kernel(x, mask, Wqkv, bqkv, Wo, bo):
    out, _ = run_sharded(x, Wqkv, bqkv, Wo, bo)
    return out


# revision 34
# speedup vs baseline: 1.1854x; 1.1854x over previous
"""Causal multi-head attention (B=2, S=2048, D=1024, H=16) on 8 trn2 cores.

Sharding: batch (2-way) x head-group (4-way) = 8 cores. Each core computes
QKV projection for its batch restricted to its 4 heads, causal attention,
and a row-parallel slice of the output projection; the host sums the 4
partial outputs per batch (the all-reduce of the row-parallel Wo matmul).

Per-core kernel (Tile framework, fp16 matmul operands / fp32 PSUM accum):
  - The host ships x pre-transposed ([D, S] fp16) and the weight slices in
    fp16, so contraction dims land on SBUF partitions with plain DMAs.
  - Q,K are produced in [feat, seq] layout (rhs = x^T), V in [seq, feat]
    layout (lhsT = x^T) with an extra ones-column per head so the PV matmul
    also produces the softmax denominator.
  - Scores are computed transposed, S_T[key, q] = K_blk.T @ Q. The two heads
    of a pair live on partitions 0:64 / 64:128, so their K=64 score matmuls
    map to distinct PE row-groups and run concurrently; emission interleaves
    j2-outer/head-inner to keep the pairs adjacent.
  - exp on ScalarE (scale folded in), one activation per (head, key-block
    pair) covering the full live column range; causal staircase masking via
    gpsimd.affine_select directly on the exp output (fully-masked column
    prefixes are never read by PV and stay garbage).
  - PV: out_T[65, q] = V_aug.T @ exp(S_T), accumulated over key blocks; row
    64 is the denominator. Normalization batches both heads of a pair: one
    [2,512] reciprocal, one K=2 fp32r broadcast matmul into [128,512].
  - Wo: out[q, :] = sum_c vw_T_c.T @ Wo_c; partials leave as fp16 via
    gpsimd PSUM->SBUF copies; host sums partials and adds bo in fp32.
"""

import numpy as np
import ml_dtypes
from contextlib import ExitStack

import concourse.bass as bass
import concourse.mybir as mybir
import concourse.tile as tile
from concourse import bacc
from concourse.bass_utils import run_bass_kernel_spmd

B, S, D, H, HD = 2, 2048, 1024, 16, 64
NCORES = 8
NHG = 4                  # head groups (cores per batch)
NH = H // NHG            # 4 local heads
FQK = NH * HD * 2        # 512 local q+k features
FV = NH * HD             # 256 local v features
QB = 512                 # query block (attention outer tile)
KB = 128                 # key block
NSC = S // QB            # 4 seq chunks
R32 = mybir.dt.float32r
F16 = mybir.dt.float16
F32 = mybir.dt.float32
F8 = mybir.dt.float8e4
DR = mybir.MatmulPerfMode.DoubleRow
EXP = mybir.ActivationFunctionType.Exp
GE = mybir.AluOpType.is_ge
# Wqk is pre-scaled by WSCALE host-side so fp8e4 quantization stays out of
# the subnormal range (raw std 0.002); folded back out in the exp scale.
WSCALE = 64.0


def _build_body(ctx, tc, x_d, x8_d, wqk8_d, wv_d, bqk_d, bv_d, wo_d, out_d):
    nc = tc.nc

    const = ctx.enter_context(tc.tile_pool(name="const", bufs=1))
    wq_pool = ctx.enter_context(tc.tile_pool(name="wqp", bufs=8))
    wvp = ctx.enter_context(tc.tile_pool(name="wvp", bufs=8))
    wop = ctx.enter_context(tc.tile_pool(name="wop", bufs=2))
    xt_pool = ctx.enter_context(tc.tile_pool(name="xtp", bufs=16))
    x8_pool = ctx.enter_context(tc.tile_pool(name="x8p", bufs=16))
    qk_pool = ctx.enter_context(tc.tile_pool(name="qkp", bufs=1))
    v_pool = ctx.enter_context(tc.tile_pool(name="vp", bufs=16))
    exp_pool = ctx.enter_context(tc.tile_pool(name="ep", bufs=6))
    vw_pool = ctx.enter_context(tc.tile_pool(name="vwp", bufs=2))
    rc_pool = ctx.enter_context(tc.tile_pool(name="rcp", bufs=3))
    os_pool = ctx.enter_context(tc.tile_pool(name="osp", bufs=3))
    p1 = ctx.enter_context(tc.tile_pool(name="p1", bufs=2, space="PSUM"))
    ps = ctx.enter_context(tc.tile_pool(name="ps", bufs=2, space="PSUM"))
    po = ctx.enter_context(tc.tile_pool(name="po", bufs=2, space="PSUM"))



    # ---- weights ----
    # bqk/bv first: tiny, and the bvb broadcast matmul is the first PE
    # instruction - queued behind the bulk weights it stalls the PE stream
    bqk_sb = const.tile([128, 4], F32)
    nc.sync.dma_start(bqk_sb, bqk_d.ap().rearrange("(f p) -> p f", p=128))
    bv_sb = const.tile([1, FV], F32)
    nc.sync.dma_start(bv_sb, bv_d.ap().rearrange("(o e) -> o e", o=1))
    # fp8 DoubleRow weights: [c2][p, j, f] holds Wqk[c2*256 + j*128 + p, f]
    wqk8_sb = []
    for c2 in range(4):
        t = wq_pool.tile([128, 2, FQK], F8, name=f"wqk{c2}", tag="wqk")
        nc.sync.dma_start(
            t, wqk8_d.ap()[c2 * 128:(c2 + 1) * 128, :].rearrange(
                "p (j f) -> p j f", j=2))
        wqk8_sb.append(t)
    # x8 chunk 0 before the remaining weights: the first QK-proj matmuls
    # need only wqk8 + x8
    x8_0 = []
    for c2 in range(4):
        xt = x8_pool.tile([128, 2, QB], F8, name="x8", tag="x8")
        nc.sync.dma_start(
            xt, x8_d.ap()[c2 * 128:(c2 + 1) * 128, :].rearrange(
                "p (j s) -> p j s", j=2)[:, :, 0:QB])
        x8_0.append(xt)
    wv_sb = []
    for dc in range(8):
        t = wvp.tile([128, FV], F16, name=f"wv{dc}", tag="wv")
        nc.sync.dma_start(t, wv_d.ap()[dc * 128:(dc + 1) * 128, :])
        wv_sb.append(t)
    xT0 = []
    for dc in range(8):
        xt = xt_pool.tile([128, QB], F16, name="xt", tag="xt")
        nc.sync.dma_start(xt, x_d.ap()[dc * 128:(dc + 1) * 128, 0:QB])
        xT0.append(xt)
    wo_sb = []
    for c in range(2):
        t = wop.tile([128, D], F16, name=f"wo{c}", tag="wo")
        nc.sync.dma_start(t, wo_d.ap()[c * 128:(c + 1) * 128, :])
        wo_sb.append(t)
    # v-bias broadcast across partitions on gpsimd (SBUF->SBUF)
    bvb_sb = const.tile([128, FV], F32)
    nc.gpsimd.partition_broadcast(bvb_sb, bv_sb)

    # ---- phase B: QKV projection ----
    qkT = [qk_pool.tile([128, S], F16, name=f"qkT{f}", tag=f"qkT{f}", bufs=1)
           for f in range(4)]
    v_tiles = []

    def emit_B_dmas(sc):
        # x tiles: fp8 interleaved for QK proj, fp16 transposed for V proj
        if sc == 0:
            return x8_0, xT0
        x8 = []
        for c2 in range(4):
            xt = x8_pool.tile([128, 2, QB], F8, name="x8", tag="x8")
            nc.sync.dma_start(
                xt, x8_d.ap()[c2 * 128:(c2 + 1) * 128, :].rearrange(
                    "p (j s) -> p j s", j=2)[:, :, sc * QB:(sc + 1) * QB])
            x8.append(xt)
        xT = []
        for dc in range(8):
            xt = xt_pool.tile([128, QB], F16, name="xt", tag="xt")
            nc.sync.dma_start(
                xt, x_d.ap()[dc * 128:(dc + 1) * 128, sc * QB:(sc + 1) * QB])
            xT.append(xt)
        return x8, xT

    def make_B_groups(sc, x8, xT):
        # projection work of chunk sc as independently emittable groups, so
        # attention emission can inject them into the PE stream exactly where
        # the PE would otherwise idle waiting on ScalarE exp
        groups = []

        def qk_group(f, x8=x8, sc=sc):
            # Q,K in [feat, seq]: psum += Wqk_c2.T @ x8, fp8 DoubleRow (K=256)
            pq = p1.tile([128, QB], F32, name="pq", tag="p1")
            for c2 in range(4):
                nc.tensor.matmul(pq, wqk8_sb[c2][:, :, f * 128:(f + 1) * 128],
                                 x8[c2], start=(c2 == 0), stop=(c2 == 3),
                                 perf_mode=DR)
            nc.vector.tensor_scalar_add(
                qkT[f][:, sc * QB:(sc + 1) * QB], pq, bqk_sb[:, f:f + 1])

        def v_group(sb, xT=xT):
            # V in [seq, feat]: psum += (x^T_blk).T @ Wv_chunk + ones column
            pv = p1.tile([128, FV], F32, name="pv", tag="p1")
            for dc in range(8):
                nc.tensor.matmul(pv, xT[dc][:, sb * 128:(sb + 1) * 128],
                                 wv_sb[dc], start=(dc == 0), stop=(dc == 7))
            vt = v_pool.tile([128, NH, HD + 1], F16, name="vt", tag="vt")
            nc.vector.tensor_add(vt[:, :, 0:HD],
                                 pv.rearrange("p (h e) -> p h e", h=NH),
                                 bvb_sb.rearrange("p (h e) -> p h e", h=NH))
            nc.gpsimd.memset(vt[:, :, HD:HD + 1], 1.0)
            v_tiles.append(vt)

        for f in range(4):
            groups.append(lambda f=f: qk_group(f))
        for sb in range(4):
            groups.append(lambda sb=sb: v_group(sb))
        return groups

    def emit_C(qi, pending):
        def fill(n):
            for _ in range(n):
                if pending:
                    pending.popleft()()

        # ---- attention + output projection for query chunk qi ----
        vwT = [vw_pool.tile([128, QB], F16, name=f"vwT{c}", tag=f"vwT{c}")
               for c in range(2)]
        for hp in range(2):
            pair = (2 * hp, 2 * hp + 1)
            nkb = (qi + 1) * 4
            poh, Q, Kt = {}, {}, {}
            for h in pair:
                poh[h] = po.tile([HD + 1, QB], F32, name="poh", tag="po")
                r0 = (h % 2) * 64
                Q[h] = qkT[h // 2][r0:r0 + 64, qi * QB:(qi + 1) * QB]
                Kt[h] = qkT[2 + h // 2][r0:r0 + 64, :]

            def koff(kb):
                # columns q < (kb - qi*4)*128 of a diagonal key-block are
                # fully masked: skip them in scores/exp/PV
                return max(0, (kb - qi * 4)) * KB

            for base in range(0, nkb, 2):
                diag = base >= qi * 4
                o0 = koff(base)
                psn = {h: ps.tile([128, 2 * QB], F32, name="psn", tag="ps")
                       for h in pair}
                # j2-outer / head-inner: adjacent matmuls hit distinct PE
                # row-groups (partitions 0:64 vs 64:128) and run concurrently
                for j2 in range(2):
                    kb = base + j2
                    off = koff(kb)
                    for h in pair:
                        nc.tensor.matmul(
                            psn[h][:, j2 * QB + off:(j2 + 1) * QB],
                            Kt[h][:, kb * KB:(kb + 1) * KB],
                            Q[h][:, off:QB], start=True, stop=True)
                fill(1)
                es = {}
                for h in pair:
                    e = exp_pool.tile([128, 2 * QB], F16, name="et", tag="et")
                    nc.scalar.activation(e[:, o0:2 * QB], psn[h][:, o0:2 * QB],
                                         EXP,
                                         scale=1.0 / (np.sqrt(HD) * WSCALE**2))
                    if diag:
                        # causal staircase: keep col q' >= partition k within
                        # each live [off:QB] slice (off == 128*j exactly)
                        for j2 in range(2):
                            off = koff(base + j2)
                            nc.gpsimd.affine_select(
                                out=e[:, j2 * QB + off:(j2 + 1) * QB],
                                in_=e[:, j2 * QB + off:(j2 + 1) * QB],
                                compare_op=GE, fill=0.0, base=0,
                                channel_multiplier=-1,
                                pattern=[[1, QB - off]])
                    es[h] = e
                for j2 in range(2):
                    kb = base + j2
                    off = koff(kb)
                    for h in pair:
                        nc.tensor.matmul(
                            poh[h][:, off:QB], v_tiles[kb][:, h, :],
                            es[h][:, j2 * QB + off:(j2 + 1) * QB],
                            start=(kb == 0), stop=(kb == nkb - 1))
            # normalization, both heads batched on one partition: one
            # reciprocal over [1, 2QB], then two col-tiled K=1 broadcast
            # matmuls (out partition bases 0 / 64) into one PSUM tile
            sum2 = rc_pool.tile([1, 2 * QB], F32, name="sum2", tag="sum2")
            for i, h in enumerate(pair):
                nc.vector.tensor_copy(sum2[:, i * QB:(i + 1) * QB],
                                      poh[h][HD:HD + 1, :])
            rc2 = rc_pool.tile([1, 2 * QB], F32, name="rc2", tag="rc2")
            nc.vector.reciprocal_approx_fast(rc2, sum2)
            fill(2)
            for i, h in enumerate(pair):
                bcs = rc_pool.tile([64, QB], F32, name="bcs", tag="bcs")
                nc.gpsimd.partition_broadcast(bcs, rc2[:, i * QB:(i + 1) * QB])
                nc.vector.tensor_mul(vwT[hp][i * 64:(i + 1) * 64, :],
                                     poh[h][0:HD, :], bcs)
        for ql in range(4):
            for do in range(2):
                pw = p1.tile([128, QB], F32, name="pw", tag="p1")
                for c in range(2):
                    nc.tensor.matmul(pw, vwT[c][:, ql * 128:(ql + 1) * 128],
                                     wo_sb[c][:, do * QB:(do + 1) * QB],
                                     start=(c == 0), stop=(c == 1))
                osb = os_pool.tile([128, QB], F16, name="osb", tag="osb")
                nc.vector.tensor_copy(osb, pw)
                nc.sync.dma_start(
                    out_d.ap()[qi * QB + ql * 128: qi * QB + (ql + 1) * 128,
                               do * QB:(do + 1) * QB], osb)

    # All input DMAs issue upfront (pools hold all 4 chunks); chunk 0's
    # projection runs dense (HAM warmup), later chunks' projection groups are
    # injected into attention's PE-idle slots (exp waits). Each C(qi) flushes
    # every group of chunks <= qi+1 before C(qi+1) needs them.
    from collections import deque
    xs = [emit_B_dmas(sc) for sc in range(NSC)]
    for g in make_B_groups(0, *xs[0]):
        g()
    pending = deque()
    for qi in range(NSC):
        if qi + 1 < NSC:
            pending.extend(make_B_groups(qi + 1, *xs[qi + 1]))
        emit_C(qi, pending)
        while pending:
            pending.popleft()()


_COMPILED = None


def get_compiled():
    global _COMPILED
    if _COMPILED is not None:
        return _COMPILED
    nc = bacc.Bacc("TRN2", target_bir_lowering=False, debug=False,
                   enable_asserts=False, num_devices=NCORES)
    x_d = nc.dram_tensor("x", [D, S], F16, kind="ExternalInput")
    x8_d = nc.dram_tensor("x8", [D // 2, 2 * S], F8, kind="ExternalInput")
    wqk8_d = nc.dram_tensor("wqk8", [D // 2, 2 * FQK], F8,
                            kind="ExternalInput")
    wv_d = nc.dram_tensor("wv", [D, FV], F16, kind="ExternalInput")
    bqk_d = nc.dram_tensor("bqk", [FQK], F32, kind="ExternalInput")
    bv_d = nc.dram_tensor("bv", [FV], F32, kind="ExternalInput")
    wo_d = nc.dram_tensor("wo", [FV, D], F16, kind="ExternalInput")
    out_d = nc.dram_tensor("out", [S, D], F16, kind="ExternalOutput")
    with tile.TileContext(nc) as tc:
        with ExitStack() as ctx:
            _build_body(ctx, tc, x_d, x8_d, wqk8_d, wv_d, bqk_d, bv_d, wo_d,
                        out_d)
    nc.compile()
    _COMPILED = nc
    return nc


def _pack_dr(a):
    """[D, cols] -> [D/2, 2*cols] fp8 with virtual row (p,j) of 256-chunk c2
    holding row c2*256 + j*128 + p (must match the kernel's rearrange)."""
    c = a.reshape(4, 2, 128, a.shape[1])
    c = np.ascontiguousarray(c.transpose(0, 2, 1, 3))
    return c.reshape(D // 2, 2 * a.shape[1]).astype(ml_dtypes.float8_e4m3)


def make_in_maps(x, Wqkv, bqkv, Wo):
    x = np.ascontiguousarray(np.asarray(x, dtype=np.float32))
    Wqkv = np.asarray(Wqkv, dtype=np.float32)
    bqkv = np.asarray(bqkv, dtype=np.float32)
    Wo = np.asarray(Wo, dtype=np.float32)
    in_maps = []
    xT = [np.ascontiguousarray(x[b].T) for b in range(B)]
    x8 = [_pack_dr(t) for t in xT]
    for c in range(NCORES):
        b, hg = divmod(c, NHG)
        qs = slice(hg * FV, (hg + 1) * FV)
        ks = slice(D + hg * FV, D + (hg + 1) * FV)
        vs = slice(2 * D + hg * FV, 2 * D + (hg + 1) * FV)
        wqk = np.concatenate([Wqkv[:, qs], Wqkv[:, ks]], axis=1)
        in_maps.append({
            "x": xT[b].astype(np.float16),
            "x8": x8[b],
            "wqk8": _pack_dr(wqk * WSCALE),
            "wv": np.ascontiguousarray(Wqkv[:, vs]).astype(np.float16),
            "bqk": np.ascontiguousarray(
                np.concatenate([bqkv[qs], bqkv[ks]])) * WSCALE,
            "bv": np.ascontiguousarray(bqkv[vs]),
            "wo": np.ascontiguousarray(Wo[hg * FV:(hg + 1) * FV, :]).astype(np.float16),
        })
    return in_maps


def run_sharded(x, Wqkv, bqkv, Wo, bo, **spmd_kwargs):
    nc = get_compiled()
    in_maps = make_in_maps(x, Wqkv, bqkv, Wo)
    res = run_bass_kernel_spmd(nc, in_maps, core_ids=list(range(NCORES)),
                               **spmd_kwargs)
    out = np.zeros((B, S, D), np.float32)
    for c in range(NCORES):
        out[c // NHG] += res.results[c]["out"].astype(np.float32)
    out += np.asarray(bo, dtype=np.float32)
    return out, res


def kernel(x, mask, Wqkv, bqkv, Wo, bo):
    out, _ = run_sharded(x, Wqkv, bqkv, Wo, bo)
    return out


# revision 39
# speedup vs baseline: 1.2310x; 1.0385x over previous
"""Causal multi-head attention (B=2, S=2048, D=1024, H=16) on 8 trn2 cores.

Sharding: batch (2-way) x head-group (4-way) = 8 cores. Each core computes
QKV projection for its batch restricted to its 4 heads, causal attention,
and a row-parallel slice of the output projection; the host sums the 4
partial outputs per batch (the all-reduce of the row-parallel Wo matmul).

Per-core kernel (Tile framework, fp16 matmul operands / fp32 PSUM accum):
  - The host ships x pre-transposed ([D, S] fp16) and the weight slices in
    fp16, so contraction dims land on SBUF partitions with plain DMAs.
  - Q,K are produced in [feat, seq] layout (rhs = x^T), V in [seq, feat]
    layout (lhsT = x^T) with an extra ones-column per head so the PV matmul
    also produces the softmax denominator.
  - Scores are computed transposed, S_T[key, q] = K_blk.T @ Q. The two heads
    of a pair live on partitions 0:64 / 64:128, so their K=64 score matmuls
    map to distinct PE row-groups and run concurrently; emission interleaves
    j2-outer/head-inner to keep the pairs adjacent.
  - exp on ScalarE (scale folded in), one activation per (head, key-block
    pair) covering the full live column range; causal staircase masking via
    gpsimd.affine_select directly on the exp output (fully-masked column
    prefixes are never read by PV and stay garbage).
  - PV: out_T[65, q] = V_aug.T @ exp(S_T), accumulated over key blocks; row
    64 is the denominator. Normalization batches both heads of a pair: one
    [2,512] reciprocal, one K=2 fp32r broadcast matmul into [128,512].
  - Wo: out[q, :] = sum_c vw_T_c.T @ Wo_c; partials leave as fp16 via
    gpsimd PSUM->SBUF copies; host sums partials and adds bo in fp32.
"""

import numpy as np
import ml_dtypes
from contextlib import ExitStack

import concourse.bass as bass
import concourse.mybir as mybir
import concourse.tile as tile
from concourse import bacc
from concourse.bass_utils import run_bass_kernel_spmd

B, S, D, H, HD = 2, 2048, 1024, 16, 64
NCORES = 8
NHG = 4                  # head groups (cores per batch)
NH = H // NHG            # 4 local heads
FQK = NH * HD * 2        # 512 local q+k features
FV = NH * HD             # 256 local v features
QB = 512                 # query block (attention outer tile)
KB = 128                 # key block
NSC = S // QB            # 4 seq chunks
R32 = mybir.dt.float32r
F16 = mybir.dt.float16
F32 = mybir.dt.float32
F8 = mybir.dt.float8e4
DR = mybir.MatmulPerfMode.DoubleRow
EXP = mybir.ActivationFunctionType.Exp
GE = mybir.AluOpType.is_ge
# Wqk is pre-scaled by WSCALE host-side so fp8e4 quantization stays out of
# the subnormal range (raw std 0.002); folded back out in the exp scale.
WSCALE = 64.0


def _build_body(ctx, tc, x_d, x8_d, wqk8_d, wv_d, bqk_d, bv_d, wo_d, out_d):
    nc = tc.nc

    const = ctx.enter_context(tc.tile_pool(name="const", bufs=1))
    wq_pool = ctx.enter_context(tc.tile_pool(name="wqp", bufs=8))
    wvp = ctx.enter_context(tc.tile_pool(name="wvp", bufs=8))
    wop = ctx.enter_context(tc.tile_pool(name="wop", bufs=2))
    xt_pool = ctx.enter_context(tc.tile_pool(name="xtp", bufs=16))
    x8_pool = ctx.enter_context(tc.tile_pool(name="x8p", bufs=16))
    qk_pool = ctx.enter_context(tc.tile_pool(name="qkp", bufs=1))
    v_pool = ctx.enter_context(tc.tile_pool(name="vp", bufs=16))
    exp_pool = ctx.enter_context(tc.tile_pool(name="ep", bufs=6))
    vw_pool = ctx.enter_context(tc.tile_pool(name="vwp", bufs=2))
    rc_pool = ctx.enter_context(tc.tile_pool(name="rcp", bufs=3))
    os_pool = ctx.enter_context(tc.tile_pool(name="osp", bufs=6))
    p1 = ctx.enter_context(tc.tile_pool(name="p1", bufs=2, space="PSUM"))
    ps = ctx.enter_context(tc.tile_pool(name="ps", bufs=2, space="PSUM"))
    po = ctx.enter_context(tc.tile_pool(name="po", bufs=2, space="PSUM"))



    # ---- warmup ----
    # HAM clock-gate needs ~3.4us of sustained PE activity to reach 2.4 GHz,
    # and the first exp pays a ~2.7us ACT table load. Burn both during the
    # initial DMA wait with dummy work on a zero tile.
    warm = const.tile([128, QB], F16)
    nc.gpsimd.memset(warm, 0.0)
    ed = const.tile([128, 16], F16)
    nc.scalar.activation(ed, warm[:, 0:16], EXP, scale=1.0)
    for i in range(10):
        pd = p1.tile([128, QB], F32, name="pd", tag="p1")
        nc.tensor.matmul(pd, warm[:, 0:128], warm, start=True, stop=True)

    # ---- weights ----
    # bqk/bv first: tiny, and the bvb broadcast matmul is the first PE
    # instruction - queued behind the bulk weights it stalls the PE stream
    bqk_sb = const.tile([128, 4], F32)
    nc.sync.dma_start(bqk_sb, bqk_d.ap().rearrange("(f p) -> p f", p=128))
    bv_sb = const.tile([1, FV], F32)
    nc.sync.dma_start(bv_sb, bv_d.ap().rearrange("(o e) -> o e", o=1))
    # fp8 DoubleRow weights: [c2][p, j, f] holds Wqk[c2*256 + j*128 + p, f]
    wqk8_sb = []
    for c2 in range(4):
        t = wq_pool.tile([128, 2, FQK], F8, name=f"wqk{c2}", tag="wqk")
        nc.sync.dma_start(
            t, wqk8_d.ap()[c2 * 128:(c2 + 1) * 128, :].rearrange(
                "p (j f) -> p j f", j=2))
        wqk8_sb.append(t)
    # x8 chunk 0 before the remaining weights: the first QK-proj matmuls
    # need only wqk8 + x8
    x8_0 = []
    for c2 in range(4):
        xt = x8_pool.tile([128, 2, QB], F8, name="x8", tag="x8")
        nc.sync.dma_start(
            xt, x8_d.ap()[c2 * 128:(c2 + 1) * 128, :].rearrange(
                "p (j s) -> p j s", j=2)[:, :, 0:QB])
        x8_0.append(xt)
    wv_sb = []
    for dc in range(8):
        t = wvp.tile([128, FV], F16, name=f"wv{dc}", tag="wv")
        nc.sync.dma_start(t, wv_d.ap()[dc * 128:(dc + 1) * 128, :])
        wv_sb.append(t)
    xT0 = []
    for dc in range(8):
        xt = xt_pool.tile([128, QB], F16, name="xt", tag="xt")
        nc.sync.dma_start(xt, x_d.ap()[dc * 128:(dc + 1) * 128, 0:QB])
        xT0.append(xt)
    wo_sb = []
    for c in range(2):
        t = wop.tile([128, D], F16, name=f"wo{c}", tag="wo")
        nc.sync.dma_start(t, wo_d.ap()[c * 128:(c + 1) * 128, :])
        wo_sb.append(t)
    # v-bias broadcast across partitions on gpsimd (SBUF->SBUF)
    bvb_sb = const.tile([128, FV], F32)
    nc.gpsimd.partition_broadcast(bvb_sb, bv_sb)

    # ---- phase B: QKV projection ----
    qkT = [qk_pool.tile([128, S], F16, name=f"qkT{f}", tag=f"qkT{f}", bufs=1)
           for f in range(4)]
    v_tiles = []

    def emit_B_dmas(sc):
        # x tiles: fp8 interleaved for QK proj, fp16 transposed for V proj
        if sc == 0:
            return x8_0, xT0
        x8 = []
        for c2 in range(4):
            xt = x8_pool.tile([128, 2, QB], F8, name="x8", tag="x8")
            nc.sync.dma_start(
                xt, x8_d.ap()[c2 * 128:(c2 + 1) * 128, :].rearrange(
                    "p (j s) -> p j s", j=2)[:, :, sc * QB:(sc + 1) * QB])
            x8.append(xt)
        xT = []
        for dc in range(8):
            xt = xt_pool.tile([128, QB], F16, name="xt", tag="xt")
            nc.sync.dma_start(
                xt, x_d.ap()[dc * 128:(dc + 1) * 128, sc * QB:(sc + 1) * QB])
            xT.append(xt)
        return x8, xT

    def make_B_groups(sc, x8, xT):
        # projection work of chunk sc as independently emittable groups, so
        # attention emission can inject them into the PE stream exactly where
        # the PE would otherwise idle waiting on ScalarE exp
        groups = []

        def qk_group(f, x8=x8, sc=sc):
            # Q,K in [feat, seq]: psum += Wqk_c2.T @ x8, fp8 DoubleRow (K=256)
            pq = p1.tile([128, QB], F32, name="pq", tag="p1")
            for c2 in range(4):
                nc.tensor.matmul(pq, wqk8_sb[c2][:, :, f * 128:(f + 1) * 128],
                                 x8[c2], start=(c2 == 0), stop=(c2 == 3),
                                 perf_mode=DR)
            nc.vector.tensor_scalar_add(
                qkT[f][:, sc * QB:(sc + 1) * QB], pq, bqk_sb[:, f:f + 1])

        def v_group(sb, xT=xT):
            # V in [seq, feat]: psum += (x^T_blk).T @ Wv_chunk + ones column
            pv = p1.tile([128, FV], F32, name="pv", tag="p1")
            for dc in range(8):
                nc.tensor.matmul(pv, xT[dc][:, sb * 128:(sb + 1) * 128],
                                 wv_sb[dc], start=(dc == 0), stop=(dc == 7))
            vt = v_pool.tile([128, NH, HD + 1], F16, name="vt", tag="vt")
            nc.vector.tensor_add(vt[:, :, 0:HD],
                                 pv.rearrange("p (h e) -> p h e", h=NH),
                                 bvb_sb.rearrange("p (h e) -> p h e", h=NH))
            nc.gpsimd.memset(vt[:, :, HD:HD + 1], 1.0)
            v_tiles.append(vt)

        for f in range(4):
            groups.append(lambda f=f: qk_group(f))
        for sb in range(4):
            groups.append(lambda sb=sb: v_group(sb))
        return groups

    def emit_C(qi, pending):
        def fill(n):
            for _ in range(n):
                if pending:
                    pending.popleft()()

        # ---- attention + output projection for query chunk qi ----
        vwT = [vw_pool.tile([128, QB], F16, name=f"vwT{c}", tag=f"vwT{c}")
               for c in range(2)]
        for hp in range(2):
            pair = (2 * hp, 2 * hp + 1)
            nkb = (qi + 1) * 4
            poh, Q, Kt = {}, {}, {}
            for h in pair:
                poh[h] = po.tile([HD + 1, QB], F32, name="poh", tag="po")
                r0 = (h % 2) * 64
                Q[h] = qkT[h // 2][r0:r0 + 64, qi * QB:(qi + 1) * QB]
                Kt[h] = qkT[2 + h // 2][r0:r0 + 64, :]

            def koff(kb):
                # columns q < (kb - qi*4)*128 of a diagonal key-block are
                # fully masked: skip them in scores/exp/PV
                return max(0, (kb - qi * 4)) * KB

            for base in range(0, nkb, 2):
                diag = base >= qi * 4
                o0 = koff(base)
                psn = {h: ps.tile([128, 2 * QB], F32, name="psn", tag="ps")
                       for h in pair}
                # j2-outer / head-inner: adjacent matmuls hit distinct PE
                # row-groups (partitions 0:64 vs 64:128) and run concurrently
                for j2 in range(2):
                    kb = base + j2
                    off = koff(kb)
                    for h in pair:
                        nc.tensor.matmul(
                            psn[h][:, j2 * QB + off:(j2 + 1) * QB],
                            Kt[h][:, kb * KB:(kb + 1) * KB],
                            Q[h][:, off:QB], start=True, stop=True)
                fill(1)
                es = {}
                for h in pair:
                    e = exp_pool.tile([128, 2 * QB], F16, name="et", tag="et")
                    nc.scalar.activation(e[:, o0:2 * QB], psn[h][:, o0:2 * QB],
                                         EXP,
                                         scale=1.0 / (np.sqrt(HD) * WSCALE**2))
                    if diag:
                        # causal staircase: keep col q' >= partition k within
                        # each live [off:QB] slice (off == 128*j exactly)
                        for j2 in range(2):
                            off = koff(base + j2)
                            nc.gpsimd.affine_select(
                                out=e[:, j2 * QB + off:(j2 + 1) * QB],
                                in_=e[:, j2 * QB + off:(j2 + 1) * QB],
                                compare_op=GE, fill=0.0, base=0,
                                channel_multiplier=-1,
                                pattern=[[1, QB - off]])
                    es[h] = e
                for j2 in range(2):
                    kb = base + j2
                    off = koff(kb)
                    for h in pair:
                        nc.tensor.matmul(
                            poh[h][:, off:QB], v_tiles[kb][:, h, :],
                            es[h][:, j2 * QB + off:(j2 + 1) * QB],
                            start=(kb == 0), stop=(kb == nkb - 1))
            # normalization per head, pipelined: copy denominator row out of
            # PSUM, reciprocal, gpsimd partition-broadcast, scale the head
            fill(2)
            for i, h in enumerate(pair):
                sumh = rc_pool.tile([1, QB], F32, name="sumh", tag="sumh")
                nc.vector.tensor_copy(sumh, poh[h][HD:HD + 1, :])
                rch = rc_pool.tile([1, QB], F32, name="rch", tag="rch")
                nc.vector.reciprocal_approx_fast(rch, sumh)
                bcs = rc_pool.tile([64, QB], F32, name="bcs", tag="bcs")
                nc.gpsimd.partition_broadcast(bcs, rch)
                nc.vector.tensor_mul(vwT[hp][i * 64:(i + 1) * 64, :],
                                     poh[h][0:HD, :], bcs)
        for ql in range(4):
            for do in range(2):
                pw = p1.tile([128, QB], F32, name="pw", tag="p1")
                for c in range(2):
                    nc.tensor.matmul(pw, vwT[c][:, ql * 128:(ql + 1) * 128],
                                     wo_sb[c][:, do * QB:(do + 1) * QB],
                                     start=(c == 0), stop=(c == 1))
                osb = os_pool.tile([128, QB], F16, name="osb", tag="osb")
                # ScalarE is idle by the final chunk's output drain; share it
                if qi == NSC - 1 and do == 1:
                    nc.scalar.copy(osb, pw)
                else:
                    nc.vector.tensor_copy(osb, pw)
                nc.sync.dma_start(
                    out_d.ap()[qi * QB + ql * 128: qi * QB + (ql + 1) * 128,
                               do * QB:(do + 1) * QB], osb)

    # All input DMAs issue upfront (pools hold all 4 chunks); chunk 0's
    # projection runs dense (HAM warmup), later chunks' projection groups are
    # injected into attention's PE-idle slots (exp waits). Each C(qi) flushes
    # every group of chunks <= qi+1 before C(qi+1) needs them.
    from collections import deque
    for g in make_B_groups(0, *emit_B_dmas(0)):
        g()
    pending = deque()
    for qi in range(NSC):
        if qi + 1 < NSC:
            pending.extend(make_B_groups(qi + 1, *emit_B_dmas(qi + 1)))
        emit_C(qi, pending)
        while pending:
            pending.popleft()()


_COMPILED = None


def get_compiled():
    global _COMPILED
    if _COMPILED is not None:
        return _COMPILED
    nc = bacc.Bacc("TRN2", target_bir_lowering=False, debug=False,
                   enable_asserts=False, num_devices=NCORES)
    x_d = nc.dram_tensor("x", [D, S], F16, kind="ExternalInput")
    x8_d = nc.dram_tensor("x8", [D // 2, 2 * S], F8, kind="ExternalInput")
    wqk8_d = nc.dram_tensor("wqk8", [D // 2, 2 * FQK], F8,
                            kind="ExternalInput")
    wv_d = nc.dram_tensor("wv", [D, FV], F16, kind="ExternalInput")
    bqk_d = nc.dram_tensor("bqk", [FQK], F32, kind="ExternalInput")
    bv_d = nc.dram_tensor("bv", [FV], F32, kind="ExternalInput")
    wo_d = nc.dram_tensor("wo", [FV, D], F16, kind="ExternalInput")
    out_d = nc.dram_tensor("out", [S, D], F16, kind="ExternalOutput")
    with tile.TileContext(nc) as tc:
        with ExitStack() as ctx:
            _build_body(ctx, tc, x_d, x8_d, wqk8_d, wv_d, bqk_d, bv_d, wo_d,
                        out_d)
    nc.compile()
    _COMPILED = nc
    return nc


def _pack_dr(a):
    """[D, cols] -> [D/2, 2*cols] fp8 with virtual row (p,j) of 256-chunk c2
    holding row c2*256 + j*128 + p (must match the kernel's rearrange)."""
    c = a.reshape(4, 2, 128, a.shape[1])
    c = np.ascontiguousarray(c.transpose(0, 2, 1, 3))
    return c.reshape(D // 2, 2 * a.shape[1]).astype(ml_dtypes.float8_e4m3)


def make_in_maps(x, Wqkv, bqkv, Wo):
    x = np.ascontiguousarray(np.asarray(x, dtype=np.float32))
    Wqkv = np.asarray(Wqkv, dtype=np.float32)
    bqkv = np.asarray(bqkv, dtype=np.float32)
    Wo = np.asarray(Wo, dtype=np.float32)
    in_maps = []
    xT = [np.ascontiguousarray(x[b].T) for b in range(B)]
    x8 = [_pack_dr(t) for t in xT]
    for c in range(NCORES):
        b, hg = divmod(c, NHG)
        qs = slice(hg * FV, (hg + 1) * FV)
        ks = slice(D + hg * FV, D + (hg + 1) * FV)
        vs = slice(2 * D + hg * FV, 2 * D + (hg + 1) * FV)
        wqk = np.concatenate([Wqkv[:, qs], Wqkv[:, ks]], axis=1)
        in_maps.append({
            "x": xT[b].astype(np.float16),
            "x8": x8[b],
            "wqk8": _pack_dr(wqk * WSCALE),
            "wv": np.ascontiguousarray(Wqkv[:, vs]).astype(np.float16),
            "bqk": np.ascontiguousarray(
                np.concatenate([bqkv[qs], bqkv[ks]])) * WSCALE,
            "bv": np.ascontiguousarray(bqkv[vs]),
            "wo": np.ascontiguousarray(Wo[hg * FV:(hg + 1) * FV, :]).astype(np.float16),
        })
    return in_maps


def run_sharded(x, Wqkv, bqkv, Wo, bo, **spmd_kwargs):
    nc = get_compiled()
    in_maps = make_in_maps(x, Wqkv, bqkv, Wo)
    res = run_bass_kernel_spmd(nc, in_maps, core_ids=list(range(NCORES)),
                               **spmd_kwargs)
    out = np.zeros((B, S, D), np.float32)
    for c in range(NCORES):
        out[c // NHG] += res.results[c]["out"].astype(np.float32)
    out += np.asarray(bo, dtype=np.float32)
    return out, res


def kernel(x, mask, Wqkv, bqkv, Wo, bo):
    out, _ = run_sharded(x, Wqkv, bqkv, Wo, bo)
    return out
